# revision 1
# baseline (speedup 1.0000x reference)
"""Trainium2 Bass kernel for nn_ArchDecoder: two stacked LSTMs (H=2048, H=4096)
unrolled DEPTH=12 sequential steps, batch=1, tensor-parallel across 8 NeuronCores.

Sharding: core k owns h_a positions [256k,256k+256), h_sum positions [256k,+256),
and the HP-LSTM state slice = union of those two regions (so prev_h_hp is local).
Weights are pre-permuted/sharded on host, cast to bf16, and
stay SBUF-resident across all 12 steps. Per step ONE merged AllGather exchanges
[c_hp | h_hp | hp-logit-partials] of step t plus [h_a | arch-logit-partials] of
step t+1 (the arch recurrence is self-contained, so it runs one step ahead).
Output-layer biases are folded in as b/8 per logit partial. Payloads are
partition-major [128, C] blocks so every DMA is the canonical spray pattern; the
gathered rank blocks are read back as one [128, 8*C] tile and the matvec rhs
chunks index its strided columns directly.
"""
import sys

for _p in ("/opt/trn_rl_repo", "/root/.axon_site", "/root/.axon_site/_ro/pypackages"):
    if _p not in sys.path:
        sys.path.insert(0, _p)

import numpy as np
import ml_dtypes

import concourse.bass as bass
import concourse.bacc as bacc
import concourse.mybir as mybir
import concourse.tile as tile
from concourse import bass_isa
from concourse.bass_utils import run_bass_kernel_spmd

NC = 8
V = 256
HA = 2048
HHP = 4096
DEPTH = 12
BF = mybir.dt.bfloat16
F32 = mybir.dt.float32
FP8 = mybir.dt.float8e4
WSC = 32.0   # fp8 weight scale for W_hh_hp
ASC = 16.0   # fp8 activation scale for c_hp
AF = mybir.ActivationFunctionType

SA = HA // NC          # 256 h_a positions per core
SS = HA // NC          # 256 h_sum positions per core
SHP = SA + SS          # 512 hp-state positions per core
MA = 4 * SA // 128     # 8  M-tiles for arch gates
MHP = 4 * SHP // 128   # 16 M-tiles for hp gates
KA = (V + HA) // 128   # 18 K-chunks for arch gates ([a ; h_a])
KHP_C = HHP // 128     # 32 c_hp K-chunks
KHP_I = (2 * V) // 128 # 4 inp K-chunks
KSUM = HHP // 128      # 32
CM = 14                # merged payload cols: [c(4) | h(4) | lpB(2) | h_a(2) | lpA(2)]


def _build_nc():
    nc = bacc.Bacc(None, target_bir_lowering=False, num_devices=NC)

    wa_e = nc.declare_dram_parameter("wa", [128, MA * KA * 128], BF, isOutput=False)
    wsum_e = nc.declare_dram_parameter("wsum", [128, 2 * KSUM * 128], BF, isOutput=False)
    whpc_e = nc.declare_dram_parameter("whpc", [128, MHP * KHP_C * 128], BF, isOutput=False)
    whpi_e = nc.declare_dram_parameter("whpi", [128, MHP * KHP_I * 128], BF, isOutput=False)
    woa_e = nc.declare_dram_parameter("woa", [128, 2 * 2 * 128], BF, isOutput=False)
    wohp_e = nc.declare_dram_parameter("wohp", [128, 2 * 4 * 128], BF, isOutput=False)
    ba_e = nc.declare_dram_parameter("ba", [128, MA], F32, isOutput=False)
    bsum_e = nc.declare_dram_parameter("bsum", [128, 2], F32, isOutput=False)
    bhp_e = nc.declare_dram_parameter("bhp", [128, MHP], F32, isOutput=False)
    boa8_e = nc.declare_dram_parameter("boa8", [128, 2], F32, isOutput=False)
    bohp8_e = nc.declare_dram_parameter("bohp8", [128, 2], F32, isOutput=False)
    init0_e = nc.declare_dram_parameter("init0", [128, NC * CM], BF, isOutput=False)
    out_e = nc.declare_dram_parameter("out", [2, DEPTH, V], F32, isOutput=True)

    with tile.TileContext(nc, num_cores=NC) as tc:
        with (
            tc.tile_pool(name="wpool", bufs=1) as wpool,
            tc.tile_pool(name="cpool", bufs=1) as cpool,
            tc.tile_pool(name="spool", bufs=3) as spool,
            tc.tile_pool(name="xpool", bufs=3) as xpool,
            tc.tile_pool(name="psA", bufs=2, space="PSUM") as psA,
            tc.tile_pool(name="psHP", bufs=2, space="PSUM") as psHP,
            tc.tile_pool(name="psHP2", bufs=2, space="PSUM") as psHP2,
            tc.tile_pool(name="psM", bufs=2, space="PSUM") as psM,
            tc.tile_pool(name="dram", bufs=2, space="DRAM") as dram,
        ):
            wa = wpool.tile([128, MA * KA * 128], BF, tag="wa")
            wsum = wpool.tile([128, 2 * KSUM * 128], BF, tag="wsum")
            whpc = wpool.tile([128, MHP * KHP_C * 128], BF, tag="whpc")
            whpi = wpool.tile([128, MHP * KHP_I * 128], BF, tag="whpi")
            woa = wpool.tile([128, 2 * 2 * 128], BF, tag="woa")
            wohp = wpool.tile([128, 2 * 4 * 128], BF, tag="wohp")
            nc.sync.dma_start(wa[:], wa_e[:])
            nc.sync.dma_start(wsum[:], wsum_e[:])
            nc.sync.dma_start(whpc[:], whpc_e[:])
            nc.sync.dma_start(whpi[:], whpi_e[:])
            nc.sync.dma_start(woa[:], woa_e[:])
            nc.sync.dma_start(wohp[:], wohp_e[:])
            ba = cpool.tile([128, MA], F32, tag="ba")
            bsum = cpool.tile([128, 2], F32, tag="bsum")
            bhp = cpool.tile([128, MHP], F32, tag="bhp")
            boa8 = cpool.tile([128, 2], F32, tag="boa8")
            bohp8 = cpool.tile([128, 2], F32, tag="bohp8")
            nc.sync.dma_start(ba[:], ba_e[:])
            nc.sync.dma_start(bsum[:], bsum_e[:])
            nc.sync.dma_start(bhp[:], bhp_e[:])
            nc.sync.dma_start(boa8[:], boa8_e[:])
            nc.sync.dma_start(bohp8[:], bohp8_e[:])

            ones_c = cpool.tile([128, 1], F32, tag="ones_c")
            ones_r = cpool.tile([1, 128], F32, tag="ones_r")
            nc.vector.memset(ones_c[:], 1.0)
            nc.vector.memset(ones_r[:], 1.0)
            c_a = cpool.tile([128, 2], F32, tag="c_a")
            nc.vector.memset(c_a[:], 0.0)
            outA = cpool.tile([128, 2 * DEPTH], F32, tag="outA")
            outHP = cpool.tile([128, 2 * DEPTH], F32, tag="outHP")

            a_bf0 = xpool.tile([128, 2], BF, tag="a_bf")
            ahp_bf0 = xpool.tile([128, 2], BF, tag="ahp_bf")
            nc.vector.memset(a_bf0[:], 1.0 / V)
            nc.vector.memset(ahp_bf0[:], 1.0 / V)
            init0 = xpool.tile([128, NC * CM], BF, tag="allM")
            nc.sync.dma_start(init0[:], init0_e[:])

            ccol = lambda kc: (kc // 4) * CM + (kc % 4)
            hcol = lambda kc: (kc // 4) * CM + 4 + (kc % 4)
            acol = lambda j: (j // 2) * CM + 10 + (j % 2)

            Ast = {}
            Bst = {}

            def A_mm(t, a_bf, allM):
                ga_ps = psA.tile([128, MA], F32, tag="ga_ps")
                order = list(range(2, KA)) + [0, 1]   # h_a chunks first, a chunks last
                for m in range(MA):
                    for kc in order:
                        if kc < 2:
                            rhs = a_bf[:, kc:kc + 1]
                        else:
                            c = acol(kc - 2)
                            rhs = allM[:, c:c + 1]
                        nc.tensor.matmul(
                            ga_ps[:, m:m + 1],
                            wa[:, (m * KA + kc) * 128:(m * KA + kc + 1) * 128],
                            rhs, start=(kc == 2), stop=(kc == 1),
                        )
                Ast[('ga', t)] = ga_ps

            def A_acts(t):
                ga_ps = Ast.pop(('ga', t))
                acts_a = spool.tile([128, MA], F32, tag="acts_a")
                ga_sb = spool.tile([128, MA], F32, tag="ga_sb")
                nc.vector.tensor_add(ga_sb[:], ga_ps[:], ba[:])
                nc.scalar.activation(acts_a[:, 0:2], ga_sb[:, 0:2], AF.Sigmoid)
                nc.scalar.activation(acts_a[:, 2:4], ga_sb[:, 2:4], AF.Sigmoid)
                nc.scalar.activation(acts_a[:, 6:8], ga_sb[:, 6:8], AF.Sigmoid)
                nc.scalar.activation(acts_a[:, 4:6], ga_sb[:, 4:6], AF.Tanh)
                t1a = spool.tile([128, 2], F32, tag="t1a")
                t2a = spool.tile([128, 2], F32, tag="t2a")
                nc.vector.tensor_mul(t1a[:], acts_a[:, 0:2], acts_a[:, 4:6])
                nc.vector.tensor_mul(t2a[:], acts_a[:, 2:4], c_a[:])
                nc.vector.tensor_add(c_a[:], t1a[:], t2a[:])
                tc_a = spool.tile([128, 2], F32, tag="tc_a")
                nc.scalar.activation(tc_a[:], c_a[:], AF.Tanh)
                ha_f = spool.tile([128, 2], F32, tag="ha_f")
                nc.vector.tensor_mul(ha_f[:], acts_a[:, 6:8], tc_a[:])
                Ast[t] = ha_f
                return ha_f

            def A_fill(t, payM):
                """Write A(t)'s sections into payM cols 10:14."""
                ha_f = Ast[t]
                nc.vector.tensor_copy(payM[:, 10:12], ha_f[:])
                la_ps = psM.tile([128, 2], F32, tag="psM")
                for m in range(2):
                    for kc in range(2):
                        nc.tensor.matmul(
                            la_ps[:, m:m + 1],
                            woa[:, (m * 2 + kc) * 128:(m * 2 + kc + 1) * 128],
                            payM[:, 10 + kc:11 + kc], start=(kc == 0), stop=(kc == 1),
                        )
                nc.vector.tensor_add(payM[:, 12:14], la_ps[:], boa8[:])

            def do_gather(payM, last=False):
                ccM = dram.tile([128, CM], BF, tag="ccM")
                gM = dram.tile([NC, 128, CM], BF, tag="gM")
                nc.sync.dma_start(ccM[:], payM[:])
                nc.gpsimd.collective_compute(
                    "AllGather", mybir.AluOpType.bypass,
                    replica_groups=[list(range(NC))],
                    ins=[ccM.opt()], outs=[gM.opt()],
                )
                allM = xpool.tile([128, NC * CM], BF, tag="allM")
                nc.scalar.dma_start(allM[:], gM[:].rearrange("r p c -> p r c"))
                return allM

            def A_soft(t, allM):
                la_red = spool.tile([128, 2], F32, tag="la_red")
                nc.vector.tensor_reduce(
                    la_red[:],
                    allM[:].rearrange("p (r c) -> p c r", r=NC)[:, 12:14, :],
                    mybir.AxisListType.X, mybir.AluOpType.add,
                )
                exp_a = spool.tile([128, 2], F32, tag="exp_a")
                nc.scalar.activation(exp_a[:], la_red[:], AF.Exp)
                sum_a = spool.tile([128, 2], F32, tag="sum_a")
                nc.gpsimd.partition_all_reduce(sum_a[:], exp_a[:], 128, bass_isa.ReduceOp.add)
                tot_a = spool.tile([128, 1], F32, tag="tot_a")
                nc.vector.tensor_add(tot_a[:], sum_a[:, 0:1], sum_a[:, 1:2])
                rcp_a = spool.tile([128, 1], F32, tag="rcp_a")
                nc.vector.reciprocal(rcp_a[:], tot_a[:])
                nc.vector.tensor_scalar_mul(outA[:, 2 * t:2 * t + 2], exp_a[:], rcp_a[:, 0:1])
                a_bf = xpool.tile([128, 2], BF, tag="a_bf")
                nc.vector.tensor_copy(a_bf[:], outA[:, 2 * t:2 * t + 2])
                return a_bf

            def B_early(t, allM):
                hs_ps = psM.tile([128, 2], F32, tag="psM")
                for m in range(2):
                    for kc in range(KSUM):
                        c = hcol(kc)
                        nc.tensor.matmul(
                            hs_ps[:, m:m + 1],
                            wsum[:, (m * KSUM + kc) * 128:(m * KSUM + kc + 1) * 128],
                            allM[:, c:c + 1], start=(kc == 0), stop=(kc == KSUM - 1),
                        )
                hs_sb = spool.tile([128, 2], F32, tag="hs_sb")
                nc.scalar.activation(hs_sb[:, 0:1], hs_ps[:, 0:1], AF.Relu, bias=bsum[:, 0:1])
                nc.scalar.activation(hs_sb[:, 1:2], hs_ps[:, 1:2], AF.Relu, bias=bsum[:, 1:2])
                ghp_ps = None
                if t > 0:
                    ghp_ps = psHP.tile([128, MHP], F32, tag="ghp_ps")
                    for m in range(MHP):
                        for kc in range(KHP_C):
                            c = ccol(kc)
                            nc.tensor.matmul(
                                ghp_ps[:, m:m + 1],
                                whpc[:, (m * KHP_C + kc) * 128:(m * KHP_C + kc + 1) * 128],
                                allM[:, c:c + 1], start=(kc == 0), stop=(kc == KHP_C - 1),
                            )
                Bst[('early', t)] = (hs_sb, ghp_ps)

            def B_inp(t, a_bf, ahp_bf):
                ghp2_ps = psHP2.tile([128, MHP], F32, tag="ghp2_ps")
                for m in range(MHP):
                    for j in (2, 3, 0, 1):   # a_hp chunks first, a chunks last
                        rhs = a_bf[:, j:j + 1] if j < 2 else ahp_bf[:, j - 2:j - 1]
                        nc.tensor.matmul(
                            ghp2_ps[:, m:m + 1],
                            whpi[:, (m * KHP_I + j) * 128:(m * KHP_I + j + 1) * 128],
                            rhs, start=(j == 2), stop=(j == 1),
                        )
                Bst[('inp', t)] = ghp2_ps

            def B_rest(t):
                """HP acts + payload cols 0:10; returns payM."""
                hs_sb, ghp_ps = Bst.pop(('early', t))
                ghp2_ps = Bst.pop(('inp', t))
                ha_f = Ast.pop(t)
                ghp_sb = spool.tile([128, MHP], F32, tag="ghp_sb")
                acts_h = spool.tile([128, MHP], F32, tag="acts_h")
                if t > 0:
                    gsum = spool.tile([128, MHP], F32, tag="gsum")
                    nc.vector.tensor_add(gsum[:], ghp_ps[:], bhp[:])
                    nc.vector.tensor_add(ghp_sb[:], gsum[:], ghp2_ps[:])
                else:
                    nc.vector.tensor_add(ghp_sb[:], ghp2_ps[:], bhp[:])
                nc.scalar.activation(acts_h[:, 0:4], ghp_sb[:, 0:4], AF.Sigmoid)
                nc.scalar.activation(acts_h[:, 4:8], ghp_sb[:, 4:8], AF.Sigmoid)
                nc.scalar.activation(acts_h[:, 12:16], ghp_sb[:, 12:16], AF.Sigmoid)
                nc.scalar.activation(acts_h[:, 8:12], ghp_sb[:, 8:12], AF.Tanh)
                t1h = spool.tile([128, 4], F32, tag="t1h")
                t2h = spool.tile([128, 4], F32, tag="t2h")
                nc.vector.tensor_mul(t1h[:], acts_h[:, 0:4], acts_h[:, 8:12])
                nc.vector.tensor_mul(t2h[:, 0:2], acts_h[:, 4:6], ha_f[:])
                nc.vector.tensor_mul(t2h[:, 2:4], acts_h[:, 6:8], hs_sb[:])
                con_f = spool.tile([128, 4], F32, tag="con_f")
                nc.vector.tensor_add(con_f[:], t1h[:], t2h[:])
                payM = spool.tile([128, CM], BF, tag="payM")
                nc.vector.tensor_copy(payM[:, 0:4], con_f[:])
                tc_h = spool.tile([128, 4], F32, tag="tc_h")
                nc.scalar.activation(tc_h[:], con_f[:], AF.Tanh)
                nc.vector.tensor_mul(payM[:, 4:8], acts_h[:, 12:16], tc_h[:])
                lhp_ps = psM.tile([128, 2], F32, tag="psM")
                for m in range(2):
                    for kc in range(4):
                        nc.tensor.matmul(
                            lhp_ps[:, m:m + 1],
                            wohp[:, (m * 4 + kc) * 128:(m * 4 + kc + 1) * 128],
                            payM[:, 4 + kc:5 + kc], start=(kc == 0), stop=(kc == 3),
                        )
                nc.vector.tensor_add(payM[:, 8:10], lhp_ps[:], bohp8[:])
                return payM

            def B_soft(t, allM):
                lh_red = spool.tile([128, 2], F32, tag="lh_red")
                nc.vector.tensor_reduce(
                    lh_red[:],
                    allM[:].rearrange("p (r c) -> p c r", r=NC)[:, 8:10, :],
                    mybir.AxisListType.X, mybir.AluOpType.add,
                )
                exp_h = spool.tile([128, 2], F32, tag="exp_h")
                nc.scalar.activation(exp_h[:], lh_red[:], AF.Exp)
                sum_h = spool.tile([128, 2], F32, tag="sum_h")
                nc.gpsimd.partition_all_reduce(sum_h[:], exp_h[:], 128, bass_isa.ReduceOp.add)
                tot_h = spool.tile([128, 1], F32, tag="tot_h")
                nc.vector.tensor_add(tot_h[:], sum_h[:, 0:1], sum_h[:, 1:2])
                rcp_h = spool.tile([128, 1], F32, tag="rcp_h")
                nc.vector.reciprocal(rcp_h[:], tot_h[:])
                nc.vector.tensor_scalar_mul(outHP[:, 2 * t:2 * t + 2], exp_h[:], rcp_h[:, 0:1])
                ahp_bf = xpool.tile([128, 2], BF, tag="ahp_bf")
                nc.vector.tensor_copy(ahp_bf[:], outHP[:, 2 * t:2 * t + 2])
                return ahp_bf

            # ---- pipelined emission: one merged gather per step; A one step ahead ----
            A_mm(0, a_bf0, init0)
            A_acts(0)
            payA0 = spool.tile([128, CM], BF, tag="payM")
            A_fill(0, payA0)
            Mprev = do_gather(payA0, last=True)   # delivers allA(0); no B sections
            B_early(0, init0)
            abf = A_soft(0, Mprev)
            B_inp(0, abf, ahp_bf0)
            payM = B_rest(0)
            A_mm(1, abf, Mprev)
            A_acts(1)
            A_fill(1, payM)
            M = do_gather(payM)                    # M[0]: allB(0) + allA(1)
            Mprev = M
            for t in range(1, DEPTH):
                ahp_bf = B_soft(t - 1, Mprev)
                B_early(t, Mprev)
                abf = A_soft(t, Mprev)
                B_inp(t, abf, ahp_bf)
                payM = B_rest(t)
                if t + 1 < DEPTH:
                    A_mm(t + 1, abf, Mprev)
                    A_acts(t + 1)
                    A_fill(t + 1, payM)
                    Mprev = do_gather(payM)
                else:
                    Mprev = do_gather(payM, last=True)
            B_soft(DEPTH - 1, Mprev)

            for t in range(DEPTH):
                nc.sync.dma_start(
                    out_e[0, t].rearrange("(m p) -> p m", p=128),
                    outA[:, 2 * t:2 * t + 2],
                )
                nc.sync.dma_start(
                    out_e[1, t].rearrange("(m p) -> p m", p=128),
                    outHP[:, 2 * t:2 * t + 2],
                )
    nc.finalize()
    return nc


_NC_CACHE = None


def _get_nc():
    global _NC_CACHE
    if _NC_CACHE is None:
        _NC_CACHE = _build_nc()
    return _NC_CACHE


def _lhsT_pack(w_cat, n_m, n_k):
    """w_cat [n_m*128 rows, n_k*128 cols] -> SBUF image [128, n_m*n_k*128] where
    cols [(m*n_k+kc)*128 + j] on partition p = w_cat[m*128 + j, kc*128 + p]."""
    a = w_cat.reshape(n_m, 128, n_k, 128)           # [m, j, kc, p]
    return np.ascontiguousarray(a.transpose(3, 0, 2, 1).reshape(128, n_m * n_k * 128))


def _prep_in_maps(x_thought_vec_arch, x_thought_vec_arch_hp,
                  W_ih_a, W_hh_a, b_ih_a, b_hh_a, W_out_a, b_out_a,
                  W_sum, b_sum, W_ih_hp, W_hh_hp, b_ih_hp, b_hh_hp,
                  W_out_hp, b_out_hp):
    f32 = np.float32
    bf16 = ml_dtypes.bfloat16
    php = np.concatenate([
        np.concatenate([np.arange(SA * k, SA * (k + 1)),
                        HA + np.arange(SS * k, SS * (k + 1))])
        for k in range(NC)
    ])
    ba_full = (np.asarray(b_ih_a) + np.asarray(b_hh_a)).astype(f32)
    bhp_full = (np.asarray(b_ih_hp) + np.asarray(b_hh_hp)).astype(f32)
    ha0 = np.asarray(x_thought_vec_arch, f32).reshape(HA)
    hhp0 = np.asarray(x_thought_vec_arch_hp, f32).reshape(HHP)
    W_ih_a = np.asarray(W_ih_a, f32); W_hh_a = np.asarray(W_hh_a, f32)
    W_out_a = np.asarray(W_out_a, f32); W_sum = np.asarray(W_sum, f32)
    W_ih_hp = np.asarray(W_ih_hp, f32); W_hh_hp = np.asarray(W_hh_hp, f32)
    W_out_hp = np.asarray(W_out_hp, f32)
    b_out_a = np.asarray(b_out_a, f32); b_out_hp = np.asarray(b_out_hp, f32)
    b_sum = np.asarray(b_sum, f32)

    init0 = np.zeros((128, NC * CM), f32)
    hhp0_p = hhp0[php]
    for r in range(NC):
        for q in range(4):
            init0[:, r * CM + 4 + q] = hhp0_p[r * SHP + q * 128: r * SHP + (q + 1) * 128]
        for m in range(2):
            init0[:, r * CM + 10 + m] = ha0[r * SA + m * 128: r * SA + (m + 1) * 128]

    in_maps = []
    for k in range(NC):
        ja = np.arange(SA * k, SA * (k + 1))
        rows_a = np.concatenate([g * HA + ja for g in range(4)])
        wa_cat = np.concatenate([W_ih_a[rows_a], W_hh_a[rows_a]], axis=1)
        jhp = php[SHP * k: SHP * (k + 1)]
        rows_hp = np.concatenate([g * HHP + jhp for g in range(4)])
        whpc_cat = W_hh_hp[rows_hp][:, php]
        whpi_cat = W_ih_hp[rows_hp]
        js = np.arange(SS * k, SS * (k + 1))
        wsum_p = W_sum[js][:, php]
        woa_p = W_out_a[:, ja]
        wohp_p = W_out_hp[:, jhp]
        in_maps.append({
            "wa": _lhsT_pack(wa_cat, MA, KA).astype(bf16),
            "wsum": _lhsT_pack(wsum_p, 2, KSUM).astype(bf16),
            "whpc": _lhsT_pack(whpc_cat, MHP, KHP_C).astype(bf16),
            "whpi": _lhsT_pack(whpi_cat, MHP, KHP_I).astype(bf16),
            "woa": _lhsT_pack(woa_p, 2, 2).astype(bf16),
            "wohp": _lhsT_pack(wohp_p, 2, 4).astype(bf16),
            "ba": np.ascontiguousarray(ba_full[rows_a].reshape(MA, 128).T),
            "bsum": np.ascontiguousarray(b_sum[js].reshape(2, 128).T),
            "bhp": np.ascontiguousarray(bhp_full[rows_hp].reshape(MHP, 128).T),
            "boa8": np.ascontiguousarray((b_out_a / NC).reshape(2, 128).T),
            "bohp8": np.ascontiguousarray((b_out_hp / NC).reshape(2, 128).T),
            "init0": init0.astype(bf16),
        })
    return in_maps


def _run(in_maps, trace=False):
    nc = _get_nc()
    return run_bass_kernel_spmd(nc, in_maps, core_ids=list(range(NC)), trace=trace)


def kernel(**inputs):
    in_maps = _prep_in_maps(**{k: np.asarray(v) for k, v in inputs.items()})
    res = _run(in_maps, trace=False)
    out = np.asarray(res.results[0]["out"], np.float32)
    return out[0][None], out[1][None]


def kernel_traced(**inputs):
    """Like kernel() but with NTFF profiling; returns ((arch, arch_hp), exec_time_ns)."""
    try:
        import ntff_hook
        ntff_hook.install()
    except Exception:
        pass
    in_maps = _prep_in_maps(**{k: np.asarray(v) for k, v in inputs.items()})
    res = _run(in_maps, trace=True)
    out = np.asarray(res.results[0]["out"], np.float32)
    return (out[0][None], out[1][None]), res.exec_time_ns



# revision 6
# speedup vs baseline: 1.2672x; 1.2672x over previous
"""Trainium2 Bass kernel for nn_ArchDecoder: two stacked LSTM cells (H=2048,
H=4096) unrolled DEPTH=12 sequential steps, batch=1, tensor-parallel across 8
NeuronCores.

v2 design (vs the v1 single-merged-gather kernel):
- Two AllGathers per step: a small A-gather carrying the arch-LSTM state
  [h_a | arch-logit-partials] (the arch chain runs ~2 steps ahead so its
  gather hides under the hp matvec stream), and a B-gather carrying
  [c_hp | h_hp | hp-logit-partials].
- Deferred softmax normalization: the unnormalized exp(logits) feed the
  next step's matvecs (linear), accumulated in separate PSUM groups that are
  scaled by 1/sum at combine time; the cross-partition sum/reciprocal runs
  off the critical path.
- No Sigmoid activations: sigma(x) = 0.5*tanh(0.5x)+0.5, so exp/tanh/relu all
  live in one ACT table -> zero ACT_TABLE_LOAD per step.
- PE stream order per step: whpc (512 matvec tiles, gated only on the
  B-gather readback) -> wsum -> B_inp -> A-block(t+2) -> lhp, so the payload
  DMA fires right after the B tail and the A-block fills the collective's
  latency shadow.
- Outputs accumulate in SBUF; one contiguous [128, 48] DMA at the end
  (the v1 per-step strided output DMAs caused 4-byte-packet storms that
  stalled the tensor engine's instruction fetch for ~12us/step).
- Weight DMAs are split by matrix (whpc by k-chunk range) across several
  trigger queues so early steps overlap the 26MB weight load.

Gate packing order is [i, f, o, g] so the three sigmoids are contiguous.
"""
import sys

for _p in ("/opt/trn_rl_repo", "/root/.axon_site", "/root/.axon_site/_ro/pypackages"):
    if _p not in sys.path:
        sys.path.insert(0, _p)

import numpy as np
import ml_dtypes

import concourse.bass as bass
import concourse.bacc as bacc
import concourse.mybir as mybir
import concourse.tile as tile
from concourse import bass_isa
from concourse.bass_utils import run_bass_kernel_spmd

NC = 8
V = 256
HA = 2048
HHP = 4096
DEPTH = 12
BF = mybir.dt.bfloat16
F32 = mybir.dt.float32
AF = mybir.ActivationFunctionType
ALU = mybir.AluOpType

SA = HA // NC          # 256 h_a positions per core
SS = HA // NC          # 256 h_sum positions per core
SHP = SA + SS          # 512 hp-state positions per core
MA = 4 * SA // 128     # 8  M-tiles for arch gates
MHP = 4 * SHP // 128   # 16 M-tiles for hp gates
KA = (V + HA) // 128   # 18 K-chunks for arch gates ([a ; h_a])
KHP_C = HHP // 128     # 32 c_hp K-chunks
KHP_I = (2 * V) // 128 # 4 inp K-chunks
KSUM = HHP // 128      # 32
CA = 4                 # A payload cols: [h_a(2) | lpA(2)]
CB = 10                # B payload cols: [c(4) | h(4) | lpB(2)]
KR1 = 16               # whpc k-chunk DMA split: chunks [0,16) and [16,32)
KR2 = KHP_C - KR1

# allA col for global h_a chunk kc (0..15); allB col for hp-state chunks
_acol = lambda kc: (kc // 2) * CA + (kc % 2)
_ccol = lambda kc: (kc // 4) * CB + (kc % 4)
_hcol = lambda kc: (kc // 4) * CB + 4 + (kc % 4)


def _build_nc():
    nc = bacc.Bacc(None, target_bir_lowering=False, num_devices=NC)

    wa_e = nc.declare_dram_parameter("wa", [128, MA * KA * 128], BF, isOutput=False)
    wsum_e = nc.declare_dram_parameter("wsum", [128, 2 * KSUM * 128], BF, isOutput=False)
    whpc1_e = nc.declare_dram_parameter("whpc1", [128, MHP * KR1 * 128], BF, isOutput=False)
    whpc2_e = nc.declare_dram_parameter("whpc2", [128, MHP * KR2 * 128], BF, isOutput=False)
    whpi_e = nc.declare_dram_parameter("whpi", [128, MHP * KHP_I * 128], BF, isOutput=False)
    woa_e = nc.declare_dram_parameter("woa", [128, 2 * 2 * 128], BF, isOutput=False)
    wohp_e = nc.declare_dram_parameter("wohp", [128, 2 * 4 * 128], BF, isOutput=False)
    ba_e = nc.declare_dram_parameter("ba", [128, MA], F32, isOutput=False)
    bsum_e = nc.declare_dram_parameter("bsum", [128, 2], F32, isOutput=False)
    bhp_e = nc.declare_dram_parameter("bhp", [128, MHP], F32, isOutput=False)
    boa8_e = nc.declare_dram_parameter("boa8", [128, 2], F32, isOutput=False)
    bohp8_e = nc.declare_dram_parameter("bohp8", [128, 2], F32, isOutput=False)
    initA_e = nc.declare_dram_parameter("initA", [128, NC * CA], BF, isOutput=False)
    initB_e = nc.declare_dram_parameter("initB", [128, NC * CB], BF, isOutput=False)
    out_e = nc.declare_dram_parameter("out", [128, 4 * DEPTH], F32, isOutput=True)

    with tile.TileContext(nc, num_cores=NC) as tc:
        with (
            tc.tile_pool(name="wpool", bufs=1) as wpool,
            tc.tile_pool(name="cpool", bufs=1) as cpool,
            tc.tile_pool(name="apool", bufs=4) as apool,
            tc.tile_pool(name="bpool", bufs=4) as bpool,
            tc.tile_pool(name="spool", bufs=4) as spool,
            tc.tile_pool(name="xpool", bufs=3) as xpool,
            tc.tile_pool(name="psA", bufs=2, space="PSUM") as psA,
            tc.tile_pool(name="psGH", bufs=2, space="PSUM") as psGH,
            tc.tile_pool(name="psI", bufs=2, space="PSUM") as psI,
            tc.tile_pool(name="psS", bufs=2, space="PSUM") as psS,
            tc.tile_pool(name="dramA", bufs=3, space="DRAM") as dramA,
            tc.tile_pool(name="dramB", bufs=3, space="DRAM") as dramB,
        ):
            # ---- weights: split DMAs over several trigger queues ----
            wa = wpool.tile([128, MA * KA * 128], BF, tag="wa")
            wsum = wpool.tile([128, 2 * KSUM * 128], BF, tag="wsum")
            whpc1 = wpool.tile([128, MHP * KR1 * 128], BF, tag="whpc1")
            whpc2 = wpool.tile([128, MHP * KR2 * 128], BF, tag="whpc2")
            whpi = wpool.tile([128, MHP * KHP_I * 128], BF, tag="whpi")
            woa = wpool.tile([128, 2 * 2 * 128], BF, tag="woa")
            wohp = wpool.tile([128, 2 * 4 * 128], BF, tag="wohp")
            nc.sync.dma_start(wa[:], wa_e[:])
            nc.sync.dma_start(woa[:], woa_e[:])
            nc.sync.dma_start(wohp[:], wohp_e[:])
            nc.scalar.dma_start(wsum[:], wsum_e[:])
            nc.scalar.dma_start(whpi[:], whpi_e[:])
            nc.gpsimd.dma_start(whpc1[:], whpc1_e[:])
            nc.gpsimd.dma_start(whpc2[:], whpc2_e[:])

            ba = cpool.tile([128, MA], F32, tag="ba")
            bsum = cpool.tile([128, 2], F32, tag="bsum")
            bhp = cpool.tile([128, MHP], F32, tag="bhp")
            boa8 = cpool.tile([128, 2], F32, tag="boa8")
            bohp8 = cpool.tile([128, 2], F32, tag="bohp8")
            initA_sb = cpool.tile([128, NC * CA], BF, tag="initA")
            initB_sb = cpool.tile([128, NC * CB], BF, tag="initB")
            nc.sync.dma_start(ba[:], ba_e[:])
            nc.sync.dma_start(bsum[:], bsum_e[:])
            nc.sync.dma_start(bhp[:], bhp_e[:])
            nc.sync.dma_start(boa8[:], boa8_e[:])
            nc.sync.dma_start(bohp8[:], bohp8_e[:])
            nc.sync.dma_start(initA_sb[:], initA_e[:])
            nc.sync.dma_start(initB_sb[:], initB_e[:])

            a0_bf = cpool.tile([128, 2], BF, tag="a0_bf")
            ahp0_bf = cpool.tile([128, 2], BF, tag="ahp0_bf")
            one_s = cpool.tile([128, 1], F32, tag="one_s")
            nc.vector.memset(a0_bf[:], 1.0 / V)
            nc.vector.memset(ahp0_bf[:], 1.0 / V)
            nc.vector.memset(one_s[:], 1.0)
            c_a = cpool.tile([128, 2], F32, tag="c_a")
            nc.vector.memset(c_a[:], 0.0)
            outA = cpool.tile([128, 2 * DEPTH], F32, tag="outA")
            outHP = cpool.tile([128, 2 * DEPTH], F32, tag="outHP")

            def A_block(t, ea_bf, Ra, allA_prev, sm):
                """Arch cell t: gates from [a(t-1)-as-exp ; h_a(t-1)], acts,
                logit partials; gather payA -> allA(t). Returns (ha_f, allA)."""
                ga = psA.tile([128, 2 * MA], F32, tag="ga")
                ga1 = ga[:, 0:MA]
                ga2 = ga[:, MA:2 * MA]
                for m in range(MA):
                    for kc in range(16):
                        nc.tensor.matmul(
                            ga1[:, m:m + 1],
                            wa[:, (m * KA + 2 + kc) * 128:(m * KA + 2 + kc + 1) * 128],
                            allA_prev[:, _acol(kc):_acol(kc) + 1],
                            start=(kc == 0), stop=(kc == 15),
                        )
                for m in range(MA):
                    for j in range(2):
                        nc.tensor.matmul(
                            ga2[:, m:m + 1],
                            wa[:, (m * KA + j) * 128:(m * KA + j + 1) * 128],
                            ea_bf[:, j:j + 1],
                            start=(j == 0), stop=(j == 1),
                        )
                g2s = spool.tile([128, MA], F32, tag="g2s")
                nc.vector.tensor_scalar(g2s[:], ga2[:], Ra[:, 0:1], None, ALU.mult)
                gsb = spool.tile([128, MA], F32, tag="gsb")
                nc.vector.tensor_add(gsb[:], g2s[:], ba[:])
                nc.vector.tensor_add(gsb[:], gsb[:], ga1[:])
                # gate cols: i=[0:2] f=[2:4] o=[4:6] g=[6:8]
                th = spool.tile([128, MA], F32, tag="th")
                nc.scalar.activation(th[:, 0:6], gsb[:, 0:6], AF.Tanh, scale=0.5)
                nc.scalar.activation(th[:, 6:8], gsb[:, 6:8], AF.Tanh)
                sg = spool.tile([128, 6], F32, tag="sg")
                nc.vector.tensor_scalar(sg[:], th[:, 0:6], 0.5, 0.5, ALU.mult, ALU.add)
                t1a = spool.tile([128, 2], F32, tag="t1a")
                t2a = spool.tile([128, 2], F32, tag="t2a")
                nc.vector.tensor_mul(t1a[:], sg[:, 0:2], th[:, 6:8])
                nc.vector.tensor_mul(t2a[:], sg[:, 2:4], c_a[:])
                nc.vector.tensor_add(c_a[:], t1a[:], t2a[:])
                tc_a = spool.tile([128, 2], F32, tag="tc_a")
                nc.scalar.activation(tc_a[:], c_a[:], AF.Tanh)
                ha_f = apool.tile([128, 2], F32, tag="ha_f")
                nc.vector.tensor_mul(ha_f[:], sg[:, 4:6], tc_a[:])
                payA = apool.tile([128, CA], BF, tag="payA")
                nc.vector.tensor_copy(payA[:, 0:2], ha_f[:])
                la = sm[:, 2:4]
                for m in range(2):
                    for kc in range(2):
                        nc.tensor.matmul(
                            la[:, m:m + 1],
                            woa[:, (m * 2 + kc) * 128:(m * 2 + kc + 1) * 128],
                            payA[:, kc:kc + 1], start=(kc == 0), stop=(kc == 1),
                        )
                nc.vector.tensor_add(payA[:, 2:4], la[:], boa8[:])
                payA_d = dramA.tile([128, CA], BF, tag="payA_d")
                gA_d = dramA.tile([NC, 128, CA], BF, tag="gA_d")
                nc.sync.dma_start(payA_d[:], payA[:])
                nc.gpsimd.collective_compute(
                    "AllGather", ALU.bypass,
                    replica_groups=[list(range(NC))],
                    ins=[payA_d.opt()], outs=[gA_d.opt()],
                )
                allA = xpool.tile([128, NC * CA], BF, tag="allA")
                nc.sync.dma_start(allA[:], gA_d[:].rearrange("r p c -> p r c"))
                return ha_f, allA

            def H2(t, allA_t):
                """exp of arch logits t (unnormalized), 1/sum, outA[:, t]."""
                red = spool.tile([128, 2], F32, tag="redA")
                nc.vector.tensor_reduce(
                    red[:],
                    allA_t[:].rearrange("p (r c) -> p c r", r=NC)[:, 2:4, :],
                    mybir.AxisListType.X, ALU.add,
                )
                e_f = spool.tile([128, 2], F32, tag="eA_f")
                sp = spool.tile([128, 1], F32, tag="sA_p")
                nc.scalar.activation(e_f[:], red[:], AF.Exp, accum_out=sp[:])
                ea_bf = apool.tile([128, 2], BF, tag="ea_bf")
                nc.vector.tensor_copy(ea_bf[:], e_f[:])
                ssum = spool.tile([128, 1], F32, tag="sA_t")
                nc.gpsimd.partition_all_reduce(ssum[:], sp[:], 128, bass_isa.ReduceOp.add)
                Ra = apool.tile([128, 1], F32, tag="Ra")
                nc.vector.reciprocal(Ra[:], ssum[:])
                nc.vector.tensor_scalar(
                    outA[:, 2 * t:2 * t + 2], e_f[:], Ra[:, 0:1], None, ALU.mult)
                return ea_bf, Ra

            def H1(t, allB_t):
                """exp of hp logits t (unnormalized), 1/sum, outHP[:, t]."""
                red = spool.tile([128, 2], F32, tag="redB")
                nc.vector.tensor_reduce(
                    red[:],
                    allB_t[:].rearrange("p (r c) -> p c r", r=NC)[:, 8:10, :],
                    mybir.AxisListType.X, ALU.add,
                )
                e_f = spool.tile([128, 2], F32, tag="eB_f")
                sp = spool.tile([128, 1], F32, tag="sB_p")
                nc.scalar.activation(e_f[:], red[:], AF.Exp, accum_out=sp[:])
                ehp_bf = bpool.tile([128, 2], BF, tag="ehp_bf")
                nc.vector.tensor_copy(ehp_bf[:], e_f[:])
                ssum = spool.tile([128, 1], F32, tag="sB_t")
                nc.gpsimd.partition_all_reduce(ssum[:], sp[:], 128, bass_isa.ReduceOp.add)
                Rhp = bpool.tile([128, 1], F32, tag="Rhp")
                nc.vector.reciprocal(Rhp[:], ssum[:])
                nc.vector.tensor_scalar(
                    outHP[:, 2 * t:2 * t + 2], e_f[:], Rhp[:, 0:1], None, ALU.mult)
                return ehp_bf, Rhp

            def B_mm(t, allB_prev, ea_bf, ehp_bf, sm):
                """PE: whpc (t>0), wsum, B_inp. Returns psum tiles."""
                ghp = None
                if t > 0:
                    ghp = psGH.tile([128, MHP], F32, tag="ghp")
                    for m in range(MHP):
                        for kc in range(KHP_C):
                            w, nk, kk = (whpc1, KR1, kc) if kc < KR1 else (whpc2, KR2, kc - KR1)
                            nc.tensor.matmul(
                                ghp[:, m:m + 1],
                                w[:, (m * nk + kk) * 128:(m * nk + kk + 1) * 128],
                                allB_prev[:, _ccol(kc):_ccol(kc) + 1],
                                start=(kc == 0), stop=(kc == KHP_C - 1),
                            )
                hs = sm[:, 0:2]
                for m in range(2):
                    for kc in range(KSUM):
                        nc.tensor.matmul(
                            hs[:, m:m + 1],
                            wsum[:, (m * KSUM + kc) * 128:(m * KSUM + kc + 1) * 128],
                            allB_prev[:, _hcol(kc):_hcol(kc) + 1],
                            start=(kc == 0), stop=(kc == KSUM - 1),
                        )
                gi = psI.tile([128, 2 * MHP], F32, tag="gi")
                g2a = gi[:, 0:MHP]
                g2b = gi[:, MHP:2 * MHP]
                for m in range(MHP):
                    for j in range(2):
                        nc.tensor.matmul(
                            g2a[:, m:m + 1],
                            whpi[:, (m * KHP_I + j) * 128:(m * KHP_I + j + 1) * 128],
                            ea_bf[:, j:j + 1], start=(j == 0), stop=(j == 1),
                        )
                for m in range(MHP):
                    for j in range(2):
                        nc.tensor.matmul(
                            g2b[:, m:m + 1],
                            whpi[:, (m * KHP_I + 2 + j) * 128:(m * KHP_I + 2 + j + 1) * 128],
                            ehp_bf[:, j:j + 1], start=(j == 0), stop=(j == 1),
                        )
                return ghp, hs, g2a, g2b

            def B_tail(t, ghp, hs, g2a, g2b, Ra, Rhp, ha_f, sm):
                """hp acts + payload; gather payB -> allB(t)."""
                hs_sb = bpool.tile([128, 2], F32, tag="hs_sb")
                nc.scalar.activation(hs_sb[:, 0:1], hs[:, 0:1], AF.Relu, bias=bsum[:, 0:1])
                nc.scalar.activation(hs_sb[:, 1:2], hs[:, 1:2], AF.Relu, bias=bsum[:, 1:2])
                u1 = spool.tile([128, MHP], F32, tag="u1")
                u2 = spool.tile([128, MHP], F32, tag="u2")
                nc.vector.tensor_scalar(u1[:], g2a[:], Ra[:, 0:1], None, ALU.mult)
                nc.vector.tensor_scalar(u2[:], g2b[:], Rhp[:, 0:1], None, ALU.mult)
                gsb = spool.tile([128, MHP], F32, tag="gsb_h")
                nc.vector.tensor_add(gsb[:], u1[:], u2[:])
                nc.vector.tensor_add(gsb[:], gsb[:], bhp[:])
                if ghp is not None:
                    nc.vector.tensor_add(gsb[:], gsb[:], ghp[:])
                # gate cols: i=[0:4] f=[4:8] o=[8:12] g=[12:16]
                th = spool.tile([128, MHP], F32, tag="th_h")
                nc.scalar.activation(th[:, 0:12], gsb[:, 0:12], AF.Tanh, scale=0.5)
                nc.scalar.activation(th[:, 12:16], gsb[:, 12:16], AF.Tanh)
                sg = spool.tile([128, 12], F32, tag="sg_h")
                nc.vector.tensor_scalar(sg[:], th[:, 0:12], 0.5, 0.5, ALU.mult, ALU.add)
                t1h = spool.tile([128, 4], F32, tag="t1h")
                t2h = spool.tile([128, 4], F32, tag="t2h")
                nc.vector.tensor_mul(t1h[:], sg[:, 0:4], th[:, 12:16])
                nc.vector.tensor_mul(t2h[:, 0:2], sg[:, 4:6], ha_f[:])
                nc.vector.tensor_mul(t2h[:, 2:4], sg[:, 6:8], hs_sb[:])
                con = bpool.tile([128, 4], F32, tag="con")
                nc.vector.tensor_add(con[:], t1h[:], t2h[:])
                payB = bpool.tile([128, CB], BF, tag="payB")
                nc.vector.tensor_copy(payB[:, 0:4], con[:])
                tch = spool.tile([128, 4], F32, tag="tch")
                nc.scalar.activation(tch[:], con[:], AF.Tanh)
                nc.vector.tensor_mul(payB[:, 4:8], sg[:, 8:12], tch[:])
                lhp = sm[:, 4:6]
                for m in range(2):
                    for kc in range(4):
                        nc.tensor.matmul(
                            lhp[:, m:m + 1],
                            wohp[:, (m * 4 + kc) * 128:(m * 4 + kc + 1) * 128],
                            payB[:, 4 + kc:5 + kc], start=(kc == 0), stop=(kc == 3),
                        )
                nc.vector.tensor_add(payB[:, 8:10], lhp[:], bohp8[:])
                payB_d = dramB.tile([128, CB], BF, tag="payB_d")
                gB_d = dramB.tile([NC, 128, CB], BF, tag="gB_d")
                nc.sync.dma_start(payB_d[:], payB[:])
                nc.gpsimd.collective_compute(
                    "AllGather", ALU.bypass,
                    replica_groups=[list(range(NC))],
                    ins=[payB_d.opt()], outs=[gB_d.opt()],
                )
                allB = xpool.tile([128, NC * CB], BF, tag="allB")
                nc.sync.dma_start(allB[:], gB_d[:].rearrange("r p c -> p r c"))
                return allB

            # ---- prologue: arch chain two steps ahead ----
            haf = {}
            eaD = {}
            RaD = {}
            sm_p = psS.tile([128, 6], F32, tag="sm")
            haf[0], allA_cur = A_block(0, a0_bf, one_s, initA_sb, sm_p)
            eaD[0], RaD[0] = H2(0, allA_cur)
            sm_p = psS.tile([128, 6], F32, tag="sm")
            haf[1], allA_cur = A_block(1, eaD[0], RaD[0], allA_cur, sm_p)

            allB_prev = initB_sb
            ehp, Rhp = ahp0_bf, one_s
            for t in range(DEPTH):
                sm = psS.tile([128, 6], F32, tag="sm")
                ghp, hs, g2a, g2b = B_mm(t, allB_prev, eaD[t], ehp, sm)
                allB_new = B_tail(t, ghp, hs, g2a, g2b, RaD[t], Rhp, haf[t], sm)
                if t + 1 < DEPTH:
                    eaD[t + 1], RaD[t + 1] = H2(t + 1, allA_cur)
                if t + 2 < DEPTH:
                    haf[t + 2], allA_cur = A_block(t + 2, eaD[t + 1], RaD[t + 1], allA_cur, sm)
                if t + 1 < DEPTH:
                    ehp, Rhp = H1(t, allB_new)
                else:
                    H1(t, allB_new)
                allB_prev = allB_new

            nc.sync.dma_start(out_e[:, 0:2 * DEPTH], outA[:])
            nc.sync.dma_start(out_e[:, 2 * DEPTH:4 * DEPTH], outHP[:])
    nc.finalize()
    return nc


_NC_CACHE = None


def _get_nc():
    global _NC_CACHE
    if _NC_CACHE is None:
        _NC_CACHE = _build_nc()
    return _NC_CACHE


def _lhsT_pack(w_cat, n_m, n_k):
    """w_cat [n_m*128 rows, n_k*128 cols] -> SBUF image [128, n_m*n_k*128] where
    cols [(m*n_k+kc)*128 + j] on partition p = w_cat[m*128 + j, kc*128 + p]."""
    a = w_cat.reshape(n_m, 128, n_k, 128)           # [m, j, kc, p]
    return np.ascontiguousarray(a.transpose(3, 0, 2, 1).reshape(128, n_m * n_k * 128))


GO = (0, 1, 3, 2)  # pack gate blocks in order i, f, o, g


def _prep_in_maps(x_thought_vec_arch, x_thought_vec_arch_hp,
                  W_ih_a, W_hh_a, b_ih_a, b_hh_a, W_out_a, b_out_a,
                  W_sum, b_sum, W_ih_hp, W_hh_hp, b_ih_hp, b_hh_hp,
                  W_out_hp, b_out_hp):
    f32 = np.float32
    bf16 = ml_dtypes.bfloat16
    php = np.concatenate([
        np.concatenate([np.arange(SA * k, SA * (k + 1)),
                        HA + np.arange(SS * k, SS * (k + 1))])
        for k in range(NC)
    ])
    ba_full = (np.asarray(b_ih_a) + np.asarray(b_hh_a)).astype(f32)
    bhp_full = (np.asarray(b_ih_hp) + np.asarray(b_hh_hp)).astype(f32)
    ha0 = np.asarray(x_thought_vec_arch, f32).reshape(HA)
    hhp0 = np.asarray(x_thought_vec_arch_hp, f32).reshape(HHP)
    W_ih_a = np.asarray(W_ih_a, f32); W_hh_a = np.asarray(W_hh_a, f32)
    W_out_a = np.asarray(W_out_a, f32); W_sum = np.asarray(W_sum, f32)
    W_ih_hp = np.asarray(W_ih_hp, f32); W_hh_hp = np.asarray(W_hh_hp, f32)
    W_out_hp = np.asarray(W_out_hp, f32)
    b_out_a = np.asarray(b_out_a, f32); b_out_hp = np.asarray(b_out_hp, f32)
    b_sum = np.asarray(b_sum, f32)

    # initA: h_a chunks at rank-major cols; initB: h_hp chunks (c cols zero)
    initA = np.zeros((128, NC * CA), f32)
    initB = np.zeros((128, NC * CB), f32)
    hhp0_p = hhp0[php]
    for r in range(NC):
        for q in range(2):
            initA[:, r * CA + q] = ha0[r * SA + q * 128: r * SA + (q + 1) * 128]
        for q in range(4):
            initB[:, r * CB + 4 + q] = hhp0_p[r * SHP + q * 128: r * SHP + (q + 1) * 128]

    in_maps = []
    for k in range(NC):
        ja = np.arange(SA * k, SA * (k + 1))
        rows_a = np.concatenate([g * HA + ja for g in GO])
        wa_cat = np.concatenate([W_ih_a[rows_a], W_hh_a[rows_a]], axis=1)
        jhp = php[SHP * k: SHP * (k + 1)]
        rows_hp = np.concatenate([g * HHP + jhp for g in GO])
        whpc_cat = W_hh_hp[rows_hp][:, php]
        whpi_cat = W_ih_hp[rows_hp]
        js = np.arange(SS * k, SS * (k + 1))
        wsum_p = W_sum[js][:, php]
        woa_p = W_out_a[:, ja]
        wohp_p = W_out_hp[:, jhp]
        in_maps.append({
            "wa": _lhsT_pack(wa_cat, MA, KA).astype(bf16),
            "wsum": _lhsT_pack(wsum_p, 2, KSUM).astype(bf16),
            "whpc1": _lhsT_pack(whpc_cat[:, :KR1 * 128], MHP, KR1).astype(bf16),
            "whpc2": _lhsT_pack(whpc_cat[:, KR1 * 128:], MHP, KR2).astype(bf16),
            "whpi": _lhsT_pack(whpi_cat, MHP, KHP_I).astype(bf16),
            "woa": _lhsT_pack(woa_p, 2, 2).astype(bf16),
            "wohp": _lhsT_pack(wohp_p, 2, 4).astype(bf16),
            "ba": np.ascontiguousarray(ba_full[rows_a].reshape(MA, 128).T),
            "bsum": np.ascontiguousarray(b_sum[js].reshape(2, 128).T),
            "bhp": np.ascontiguousarray(bhp_full[rows_hp].reshape(MHP, 128).T),
            "boa8": np.ascontiguousarray((b_out_a / NC).reshape(2, 128).T),
            "bohp8": np.ascontiguousarray((b_out_hp / NC).reshape(2, 128).T),
            "initA": initA.astype(bf16),
            "initB": initB.astype(bf16),
        })
    return in_maps


def _unpack_out(out):
    """out [128, 4*DEPTH] f32 -> (arch [1, DEPTH, V], arch_hp [1, DEPTH, V])."""
    out = np.asarray(out, np.float32)
    arch = out[:, :2 * DEPTH].reshape(128, DEPTH, 2).transpose(1, 2, 0).reshape(DEPTH, V)
    ahp = out[:, 2 * DEPTH:].reshape(128, DEPTH, 2).transpose(1, 2, 0).reshape(DEPTH, V)
    return arch[None], ahp[None]


def _run(in_maps, trace=False):
    nc = _get_nc()
    return run_bass_kernel_spmd(nc, in_maps, core_ids=list(range(NC)), trace=trace)


def kernel(**inputs):
    in_maps = _prep_in_maps(**{k: np.asarray(v) for k, v in inputs.items()})
    res = _run(in_maps, trace=False)
    return _unpack_out(res.results[0]["out"])


def kernel_traced(**inputs):
    """Like kernel() but with NTFF profiling; returns ((arch, arch_hp), exec_time_ns)."""
    try:
        import ntff_hook
        ntff_hook.install()
    except Exception:
        pass
    in_maps = _prep_in_maps(**{k: np.asarray(v) for k, v in inputs.items()})
    res = _run(in_maps, trace=True)
    return _unpack_out(res.results[0]["out"]), res.exec_time_ns
